# revision 30
# baseline (speedup 1.0000x reference)
"""Trainium2 Bass kernel for LongcatFlash MoE experts (expert-parallel, 8 cores).

Problem: T=4096 tokens, H=1024, I=512, 32 routed + 8 zero (identity) experts,
top-4 routing, per-expert capacity 768.

Strategy (sharding_hint = expert parallelism):
  - Host: compute routing (stable sort by expert, capacity clip), permute
    tokens to their expert's core (the "all-to-all"), quantize activations
    and weights to fp8 e4m3 (weights pre-scaled by 64 to clear the fp8
    subnormal cutoff), build per-core packed buffers.
  - Device (8 cores, SPMD): each core owns 4 routed experts (snake-assigned
    by token count so per-slot sizes match across cores); per expert run the
    gated MLP as DoubleRow fp8 matmuls (2 rows/cycle, 2x bf16 throughput):
        gu[o, c]  = sum_h guT[h, o] * xT[h, c]      (o = 2I rows, c = tokens)
        mid[i, c] = silu(gate[i, c]/64) * (up[i, c]/64)   -> fp8
        y[h, c]   = sum_i dnT[i, h] * mid[i, c]           (psum = 64*y)
    Tokens live on the free dim; weights are the stationary operand.
  - Host: gather per-assignment outputs, scale by router_weight/64,
    scatter-add per token, add the zero-expert weighted-identity term.
"""

import os

import ml_dtypes
import numpy as np

N_CORES = 8
R = 32  # routed experts
E_PER_CORE = R // N_CORES  # 4
CAPACITY = 768
H = 1024
I_DIM = 512
Q = H // 256  # 4 h-pairs (DoubleRow consumes 256 contraction rows per MM)
R2 = I_DIM // 256  # 2 i-pairs
SC = 64.0  # weight pre-scale (host); compensated by 1/SC on device + host

# pad per-slot token counts to a multiple of PADN (PE moving-operand rate)
PADN = int(os.environ.get("MOE_PADN", "16"))
# optional small leading chunk for slot 0 (extra tensor-queue issue slots
# outweighed the DMA-latency win in measurement; off by default)
LEAD = int(os.environ.get("MOE_LEAD", "0"))

LAST_RUN = {}  # filled with exec_time_ns etc. for test harness use

F8 = ml_dtypes.float8_e4m3


def _route(idx, wts, n_tok):
    """Replicates the reference's capacity-buffer routing exactly.

    Returns per-assignment (expert, token, weight, flat_index) for kept routed
    assignments, sorted by expert (stable), plus zero-expert weights.
    """
    K = idx.shape[1]
    A = n_tok * K
    flat_e = idx.reshape(-1).astype(np.int64)
    flat_t = np.repeat(np.arange(n_tok, dtype=np.int64), K)
    flat_w = wts.reshape(-1)
    order = np.argsort(flat_e, kind="stable")
    se = flat_e[order]
    st = flat_t[order]
    sw = flat_w[order]
    counts = np.bincount(flat_e, minlength=R + 8)
    starts = np.cumsum(counts) - counts
    pos = np.arange(A, dtype=np.int64) - starts[se]
    valid = (se < R) & (pos < CAPACITY)
    zero_w = np.where(idx >= R, wts, 0.0).sum(axis=1)
    return se[valid], st[valid], sw[valid], order[valid], zero_w


def _plan(cnts):
    """Snake-assign experts to (core, slot) by descending count; common
    per-slot sizes = max count in the slot's rank group, padded to PADN."""
    order = np.argsort(-cnts, kind="stable")
    slots = np.zeros((N_CORES, E_PER_CORE), dtype=np.int64)
    for j in range(E_PER_CORE):
        grp = order[j * N_CORES : (j + 1) * N_CORES]
        cores = range(N_CORES) if j % 2 == 0 else range(N_CORES - 1, -1, -1)
        for c, e in zip(cores, grp):
            slots[c, j] = e
    sizes = []
    for j in range(E_PER_CORE):
        m = int(cnts[slots[:, j]].max())
        m = max(PADN, ((m + PADN - 1) // PADN) * PADN)
        sizes.append(m)
    return slots, tuple(sizes)


def _chunks(S, lead=0):
    """Split a slot of S tokens into PSUM-sized pieces (<=512 fp32 cols),
    optionally with a small leading piece."""
    out = []
    c0 = 0
    if 0 < lead < S:
        out.append((0, lead))
        c0 = lead
        S = S - lead
    n = (S + 511) // 512
    base = S // n
    rem = S - base * n
    for i in range(n):
        cn = base + (1 if i < rem else 0)
        out.append((c0, cn))
        c0 += cn
    return out


_BUILD_CACHE = {}


def _build_bass(sizes):
    import concourse.bacc as bacc
    import concourse.mybir as mybir
    from concourse import tile

    if sizes in _BUILD_CACHE:
        return _BUILD_CACHE[sizes]

    FT = mybir.dt.float32
    F8D = mybir.dt.float8e4
    BF = mybir.dt.bfloat16
    DR = mybir.MatmulPerfMode.DoubleRow
    silu_fn = mybir.ActivationFunctionType.Silu
    copy_fn = mybir.ActivationFunctionType.Copy
    mult = mybir.AluOpType.mult

    NP = sum(sizes)
    offs = np.cumsum([0] + list(sizes))[:-1]
    piece_lists = [_chunks(S, lead=LEAD if j == 0 else 0) for j, S in enumerate(sizes)]

    nc = bacc.Bacc(None)
    xt_ds = [
        nc.declare_dram_parameter(f"xt{j}", [128, Q, 2, S], F8D, isOutput=False)
        for j, S in enumerate(sizes)
    ]
    # gu packaged per (o-tile, gate/up) so the first PSUM group only needs one
    # 128KB part (DMA engines are slow per-transfer; many small transfers
    # engage more engines in parallel)
    gu_d = nc.declare_dram_parameter(
        "guw", [E_PER_CORE, 4, 2, 128, Q, 2, 128], F8D, isOutput=False
    )
    dn_d = nc.declare_dram_parameter(
        "dnw", [E_PER_CORE, 128, R2, 2, 1024], F8D, isOutput=False
    )
    yt_d = nc.declare_dram_parameter("yt", [128, 8 * NP], BF, isOutput=True)

    n_pieces = sum(len(p) for p in piece_lists)

    with tile.TileContext(nc) as tc:
        with (
            tc.tile_pool(name="xpool", bufs=E_PER_CORE) as xpool,
            tc.tile_pool(name="gupool", bufs=4 * E_PER_CORE) as gupool,
            tc.tile_pool(name="dnpool", bufs=E_PER_CORE) as dnpool,
            # sil/mid tiles are uniquely slotted (no reuse): keeps ACT/DVE
            # writes at a single sync-wait and avoids WAR stalls
            tc.tile_pool(name="silpool", bufs=2 * n_pieces) as silpool,
            tc.tile_pool(name="midpool", bufs=R2 * n_pieces) as midpool,
            tc.tile_pool(name="ypool", bufs=E_PER_CORE) as ypool,
            tc.tile_pool(name="gatepool", bufs=2) as gatepool,
            tc.tile_pool(name="pgpool", bufs=2, space="PSUM") as pgpool,
            tc.tile_pool(name="pupool", bufs=2, space="PSUM") as pupool,
            tc.tile_pool(name="pypool", bufs=4, space="PSUM") as pypool,
        ):
            # DMA flood control: only slot 0/1 inputs are triggered up front.
            # Slot j>=2 inputs are triggered from gpsimd behind a dummy copy
            # that reads slot (j-2)'s first silu output, so they queue only
            # once compute is underway; x1 is triggered on scalar after the
            # first silu. This keeps early HBM bandwidth for the critical
            # first-matmul working set (x0 + gu0 part 0).
            xts = [
                xpool.tile([128, Q, 2, S], F8D, tag="xt", name=f"xt{j}")
                for j, S in enumerate(sizes)
            ]
            guts = [
                [
                    [
                        gupool.tile(
                            [128, Q, 2, 128], F8D, tag="gu", name=f"gu{j}_{oi}_{g}"
                        )
                        for g in range(2)
                    ]
                    for oi in range(4)
                ]
                for j in range(E_PER_CORE)
            ]
            dnts = [
                dnpool.tile([128, R2, 2, 1024], F8D, tag="dn", name=f"dn{j}")
                for j in range(E_PER_CORE)
            ]

            def load_slot_weights(j, eng):
                for oi in range(4):
                    for g in range(2):
                        eng.dma_start(guts[j][oi][g][:], gu_d[j, oi, g])
                eng.dma_start(dnts[j][:], dn_d[j])

            # x0 split per h-pair across two engines to engage more DMA queues
            for q in range(Q):
                eng = nc.scalar if q < 2 else nc.gpsimd
                eng.dma_start(xts[0][:, q], xt_ds[0][:, q])
            load_slot_weights(0, nc.sync)
            load_slot_weights(1, nc.sync)

            copy_rr = [0, 1, 0, 1, 0, 1, 0, 1]  # h -> vector/scalar (PSUM readers)
            for j, S in enumerate(sizes):
                ywide = ypool.tile([128, 8 * S], BF, tag="yo", name=f"yw{j}")
                for pi, (c0, cn) in enumerate(piece_lists[j]):
                    mids = []
                    for r in range(R2):
                        mids.append(
                            midpool.tile(
                                [128, 2, cn], F8D, tag="mid", name=f"mid{j}_{c0}_{r}"
                            )
                        )
                    for oi in range(4):
                        pg = pgpool.tile([128, cn], FT, tag="pg")
                        pu = pupool.tile([128, cn], FT, tag="pu")
                        for q in range(Q):
                            nc.tensor.matmul(
                                pg[:],
                                guts[j][oi][0][:, q],
                                xts[j][:, q, :, c0 : c0 + cn],
                                start=(q == 0),
                                stop=(q == Q - 1),
                                perf_mode=DR,
                            )
                        for q in range(Q):
                            nc.tensor.matmul(
                                pu[:],
                                guts[j][oi][1][:, q],
                                xts[j][:, q, :, c0 : c0 + cn],
                                start=(q == 0),
                                stop=(q == Q - 1),
                                perf_mode=DR,
                            )
                        sil = silpool.tile([128, cn], FT, tag="sil")
                        nc.scalar.activation(sil[:], pg[:], silu_fn, scale=1.0 / SC)
                        if pi == 0 and oi == 0:
                            if j == 0:
                                nc.scalar.dma_start(xts[1][:], xt_ds[1][:])
                            if j < E_PER_CORE - 2:
                                # progress gate: gpsimd blocks on this copy
                                # until slot j's first silu lands, then queues
                                # slot j+2's input DMAs
                                gate = gatepool.tile(
                                    [128, 1], FT, tag="gate", name=f"gate{j}"
                                )
                                nc.gpsimd.tensor_copy(gate[:], sil[:, 0:1])
                                nc.gpsimd.dma_start(
                                    xts[j + 2][:], xt_ds[j + 2][:]
                                )
                                load_slot_weights(j + 2, nc.gpsimd)
                        nc.vector.scalar_tensor_tensor(
                            mids[oi // 2][:, oi % 2, :], pu[:], 1.0 / SC, sil[:],
                            mult, mult,
                        )
                    for h in range(8):
                        py = pypool.tile([128, cn], FT, tag="py")
                        for r in range(R2):
                            nc.tensor.matmul(
                                py[:],
                                dnts[j][:, r, :, h * 128 : (h + 1) * 128],
                                mids[r][:],
                                start=(r == 0),
                                stop=(r == R2 - 1),
                                perf_mode=DR,
                            )
                        dst = ywide[:, h * S + c0 : h * S + c0 + cn]
                        if copy_rr[h] == 0:
                            nc.vector.tensor_copy(dst, py[:])
                        else:
                            nc.scalar.activation(dst, py[:], copy_fn)
                if j < E_PER_CORE - 1:
                    nc.sync.dma_start(
                        yt_d[:, 8 * offs[j] : 8 * offs[j] + 8 * S], ywide[:]
                    )
                else:
                    # last expert: stream out per h on two engines so the
                    # tail transfers overlap the remaining copies
                    for h in range(8):
                        lo = 8 * offs[j] + h * S
                        eng = nc.sync if h % 2 == 0 else nc.scalar
                        eng.dma_start(
                            yt_d[:, lo : lo + S],
                            ywide[:, h * S : (h + 1) * S],
                        )

    nc.finalize()
    _BUILD_CACHE[sizes] = nc
    return nc


def _install_trace_shims():
    """Make trace=True usable in this image: provide the NTFF hook module and
    neutralize the artifact upload (no bucket access needed for local use)."""
    import sys
    import types

    try:
        import antenv.axon_hooks  # noqa: F401
    except ImportError:
        hook = None
        try:
            from trn_agent_boot.trn_boot import _ntff_profile_via_ctypes

            hook = _ntff_profile_via_ctypes("/opt/axon/libaxon_pjrt.so")
        except Exception:
            hook = None
        mod = types.ModuleType("antenv.axon_hooks")
        mod._hook = hook
        mod.get_axon_ntff_profile_hook = lambda: mod._hook
        mod.set_axon_ntff_profile_hook = lambda h: setattr(mod, "_hook", h)
        sys.modules["antenv.axon_hooks"] = mod

    import concourse.bass_utils as bu

    orig_upload = bu.upload_artifacts

    def safe_upload(tmpdir):
        try:
            return orig_upload(tmpdir)
        except Exception:
            return tmpdir
    bu.upload_artifacts = safe_upload


def _prep_core(c, slots, sizes, cnts, estarts, vt, hq, guq, dnq):
    """Build one core's input map (fp8, DoubleRow-packed layouts)."""
    xts = {}
    guw = np.zeros((E_PER_CORE, 4, 2, 128, Q, 2, 128), dtype=F8)
    dnw = np.zeros((E_PER_CORE, 128, R2, 2, 1024), dtype=F8)
    for j in range(E_PER_CORE):
        S = sizes[j]
        ge = slots[c, j]
        s0, cnt = estarts[ge], cnts[ge]
        xt = np.zeros((128, Q, 2, S), dtype=F8)
        if cnt:
            toks = vt[s0 : s0 + cnt]
            # [cnt, H] -> [H, cnt] -> [Q, 2, 128, cnt] -> [128, Q, 2, cnt]
            xb = hq[toks].T.reshape(Q, 2, 128, cnt).transpose(2, 0, 1, 3)
            xt[:, :, :, :cnt] = xb
        xts[f"xt{j}"] = np.ascontiguousarray(xt)
        # W [2I, H]; part (oi, 0)=gate rows oi*128.., (oi, 1)=up rows 512+..
        # block [o', H] -> [o', q, s, p] -> [p, q, s, o']
        for oi in range(4):
            gate = guq[ge][oi * 128 : (oi + 1) * 128]
            up = guq[ge][512 + oi * 128 : 512 + (oi + 1) * 128]
            guw[j, oi, 0] = gate.reshape(128, Q, 2, 128).transpose(3, 1, 2, 0)
            guw[j, oi, 1] = up.reshape(128, Q, 2, 128).transpose(3, 1, 2, 0)
        # Wdn [H, I] -> [p, r, s, h]: dnw[p, r, s, h] = Wdn[h, (2r+s)*128+p]
        Wd = dnq[ge].reshape(1024, R2, 2, 128)  # [h, r, s, p]
        dnw[j] = Wd.transpose(3, 1, 2, 0)
    return {
        **xts,
        "guw": np.ascontiguousarray(guw),
        "dnw": np.ascontiguousarray(dnw),
    }


def kernel(**inputs):
    from concourse.bass_utils import run_bass_kernel_spmd

    hidden = np.ascontiguousarray(np.asarray(inputs["hidden_states"], dtype=np.float32))
    idx = np.asarray(inputs["top_k_index"]).astype(np.int64)
    wts = np.asarray(inputs["top_k_weights"], dtype=np.float32)
    gup = np.asarray(inputs["gate_up_proj"], dtype=np.float32)
    dnp = np.asarray(inputs["down_proj"], dtype=np.float32)

    n_tok = hidden.shape[0]
    K = idx.shape[1]

    ve, vt, vw, va, zero_w = _route(idx, wts, n_tok)
    cnts = np.bincount(ve, minlength=R)
    estarts = np.cumsum(cnts) - cnts
    slots, sizes = _plan(cnts)
    NP = sum(sizes)
    offs = np.cumsum([0] + list(sizes))[:-1]

    # quantize once, globally
    hq = hidden.astype(F8)  # [T, H]
    guq = (gup[:R] * SC).astype(F8)  # [R, 2I, H]
    dnq = (dnp * SC).astype(F8)  # [R, H, I]

    in_maps = [
        _prep_core(c, slots, sizes, cnts, estarts, vt, hq, guq, dnq)
        for c in range(N_CORES)
    ]

    nc = _build_bass(sizes)

    trace = bool(int(os.environ.get("KERNEL_TRACE", "0")))
    if trace:
        _install_trace_shims()
    res = run_bass_kernel_spmd(nc, in_maps, list(range(N_CORES)), trace=trace)
    LAST_RUN["exec_time_ns"] = res.exec_time_ns
    LAST_RUN["mean_exec_time_ns"] = res.mean_exec_time_ns
    LAST_RUN["instructions_and_trace"] = res.instructions_and_trace
    LAST_RUN["profile_json"] = res.profile_json

    # ---- combine on host ----
    out = hidden * zero_w[:, None].astype(np.float32)
    acc = np.zeros((n_tok * K, H), dtype=np.float32)
    for c in range(N_CORES):
        yt = np.asarray(res.results[c]["yt"]).astype(np.float32)  # [128, 8*NP]
        for j in range(E_PER_CORE):
            ge = slots[c, j]
            s0, cnt = estarts[ge], cnts[ge]
            if cnt == 0:
                continue
            S = sizes[j]
            blk = yt[:, 8 * offs[j] : 8 * offs[j] + 8 * S]
            # [128, 8, S] -> [8, 128, S] -> [H, S]; psum held 64*y
            y = blk.reshape(128, 8, S).transpose(1, 0, 2).reshape(H, S)[:, :cnt].T
            acc[va[s0 : s0 + cnt]] = y * (vw[s0 : s0 + cnt, None] / SC)
    out += acc.reshape(n_tok, K, H).sum(axis=1)
    return out
